# revision 7
# baseline (speedup 1.0000x reference)
"""HMM forward-algorithm Bass kernel for Trainium2, SPMD over 8 NeuronCores.

Strategy (data-parallel over batch, 8 sequences/core):
 - Host prep: softmax(trans) -> A^T fp8e4 (x2^8); d = logsumexp(emis, 1);
   gather emission cols for observed tokens, exp, scale 2^15 -> fp8e4 staged
   in natural [state, b, t] layout (device DMA does the layout shuffle);
   t=0 (alpha0/m0/q0) on host.  Upload ~10.5MB total.
 - Device: 255 steps of scaled forward recursion
     pps = (A*2^8) @ q      (16 fp8-weight matmuls, PSUM fp32)
     u   = pps * ep_t       (1 DVE op, out bf16 -> next q)
     sig[t] = colsums(u)    (1 ones-vector matmul into a PSUM history bank)
     every 8 steps: q = u / sum(u)   (4 ones-matmuls + reciprocal + mult)
 - Host post: raw fp32 sigma history (255*32 floats/core) -> log + block
   prefix + per-sequence T_b select.  Per-step constant 2^23 (= 2^15 ep
   scale * 2^8 A scale) is subtracted analytically.
"""
import sys
sys.path.insert(0, "/opt/trn_rl_repo")
import numpy as np
import ml_dtypes

import concourse.bass as bass
import concourse.bacc as bacc
import concourse.mybir as mybir
import concourse.tile as tile
from concourse import bass2jax

N_CORES = 8
N = 512        # states
B = 64         # batch
TMAX = 256     # sequence length
BL = B // N_CORES       # 8 sequences per core
NT = N // 128           # 4 state tiles
NSTEP = TMAX - 1        # 255 device steps (t = 1..255)
KRE = 8                 # renormalize every KRE steps
S_EP = 15.0             # ep storage scale 2^S_EP
S_A = 8.0               # A storage scale 2^S_A
C_EFF = S_EP + S_A      # per-step constant folded into sigma
DT = mybir.dt

_CACHE = {}
LAST_EXEC_NS = None


def _build_main_kernel():
    nc = bacc.Bacc("TRN2", target_bir_lowering=False, debug=False,
                   num_devices=N_CORES)
    f32 = DT.float32
    bf16 = DT.bfloat16
    fp8 = DT.float8e4
    at_d = nc.dram_tensor("at", [N, N], fp8, kind="ExternalInput")    # A^T [k, j]
    ep_d = nc.dram_tensor("ep", [N, BL * TMAX], fp8,
                          kind="ExternalInput")                       # [(g p), (b, t)]
    q0_d = nc.dram_tensor("q0", [128, NT * BL], bf16, kind="ExternalInput")
    sig_d = nc.dram_tensor("sig", [1, NSTEP * BL], f32,
                           kind="ExternalOutput")
    MUL = mybir.AluOpType.mult

    with tile.TileContext(nc) as tc:
        with (tc.tile_pool(name="persist", bufs=1) as pp,
              tc.tile_pool(name="work", bufs=3) as wp,
              tc.tile_pool(name="pps_pool", bufs=2, space="PSUM") as psp,
              tc.tile_pool(name="sig_pool", bufs=2, space="PSUM") as sgp,
              tc.tile_pool(name="rps_pool", bufs=2, space="PSUM") as rpp):

            # ---------- persistent tiles ----------
            atk = [pp.tile([128, N], fp8, name=f"atk{kt}", tag=f"atk{kt}")
                   for kt in range(NT)]
            for kt in range(NT):
                nc.sync.dma_start(atk[kt][:], at_d.ap()[kt * 128:(kt + 1) * 128, :])
            # ep tile [p, g, b, t] incl t=0 (t=0 unused; keeps DMA APs 3-dim)
            ep = pp.tile([128, NT, BL, TMAX], fp8)
            nc.sync.dma_start(
                ep[:], ep_d.ap().rearrange("(g p) (b t) -> p g b t",
                                           g=NT, b=BL))
            onescol = pp.tile([128, 1], bf16)
            nc.gpsimd.memset(onescol[:], 1.0)
            ones128 = pp.tile([128, 128], bf16)
            nc.gpsimd.memset(ones128[:], 1.0)
            sig_hist = pp.tile([1, NSTEP * BL], f32)

            q = pp.tile([128, NT, BL], bf16, name="q0t", tag="q0t")
            nc.sync.dma_start(
                q[:], q0_d.ap().rearrange("p (g b) -> p g b", g=NT))

            # ---------- recursion ----------
            SIGBLK = 16            # sigma history steps per PSUM bank
            sps = None
            for i in range(NSTEP):
                # P = A @ q : 16 matmuls, accumulation groups sequential per jt
                pps = psp.tile([128, NT, BL], f32, tag="pps")
                for jt in range(NT):
                    for kt in range(NT):
                        nc.tensor.matmul(
                            pps[:, jt, :],
                            lhsT=atk[kt][:, jt * 128:(jt + 1) * 128],
                            rhs=q[:, kt, :],
                            start=(kt == 0), stop=(kt == NT - 1))
                # u = pps * ep_t  -> becomes next q (bf16)
                u = wp.tile([128, NT, BL], bf16, tag="u")
                nc.vector.tensor_tensor(u[:], pps[:], ep[:, :, :, i + 1], op=MUL)

                # sigma partials: [1, NT*BL] into history PSUM bank
                s_idx = i % SIGBLK
                if s_idx == 0:
                    sps = sgp.tile([1, SIGBLK * NT * BL], f32, tag="sps")
                nc.tensor.matmul(
                    sps[:, s_idx * NT * BL:(s_idx + 1) * NT * BL],
                    lhsT=onescol[:],
                    rhs=u[:].rearrange("p g b -> p (g b)"),
                    start=True, stop=True)
                if s_idx == SIGBLK - 1 or i == NSTEP - 1:
                    blk = i // SIGBLK
                    ns = s_idx + 1
                    # reduce the NT partials: [1, (s g b)] -> [1, s, b]
                    nc.vector.reduce_sum(
                        sig_hist[:, blk * SIGBLK * BL:
                                 blk * SIGBLK * BL + ns * BL]
                        .rearrange("p (s b) -> p s b", s=ns),
                        sps[:, 0:ns * NT * BL]
                        .rearrange("p (s g b) -> p s b g", s=ns, g=NT),
                        axis=mybir.AxisListType.X)

                # renormalize every KRE steps (skip on last step)
                if i % KRE == KRE - 1 and i != NSTEP - 1:
                    rps = rpp.tile([128, BL], f32, tag="rps")
                    for g in range(NT):
                        nc.tensor.matmul(rps[:], lhsT=ones128[:],
                                         rhs=u[:, g, :],
                                         start=(g == 0), stop=(g == NT - 1))
                    inv = wp.tile([128, 1, BL], f32, tag="inv")
                    nc.vector.reciprocal(inv[:, 0, :], rps[:])
                    qn = wp.tile([128, NT, BL], bf16, tag="qn")
                    nc.vector.tensor_tensor(
                        qn[:], u[:], inv[:].to_broadcast([128, NT, BL]), op=MUL)
                    q = qn
                else:
                    q = u

            nc.sync.dma_start(sig_d.ap(), sig_hist[:])
    nc.compile()
    return nc


def _get_runner():
    """Build (once) a cached jitted SPMD runner for the main kernel."""
    if "runner" in _CACHE:
        return _CACHE["runner"]
    import jax
    from jax.sharding import Mesh, PartitionSpec
    from jax.experimental.shard_map import shard_map

    nc = _CACHE["main"]
    bass2jax.install_neuronx_cc_hook()

    partition_name = (nc.partition_id_tensor.name
                      if nc.partition_id_tensor else None)
    in_names, out_names, out_avals, zero_outs = [], [], [], []
    for alloc in nc.m.functions[0].allocations:
        if not isinstance(alloc, mybir.MemoryLocationSet):
            continue
        name = alloc.memorylocations[0].name
        if alloc.kind == "ExternalInput":
            if name != partition_name:
                in_names.append(name)
        elif alloc.kind == "ExternalOutput":
            shape = tuple(alloc.tensor_shape)
            dtype = mybir.dt.np(alloc.dtype)
            out_names.append(name)
            out_avals.append(jax.core.ShapedArray(shape, dtype))
            zero_outs.append(np.zeros(shape, dtype))
    n_params = len(in_names)
    n_outs = len(out_avals)
    all_names = in_names + out_names
    if partition_name is not None:
        all_names = all_names + [partition_name]

    def _body(*args):
        operands = list(args)
        if partition_name is not None:
            operands.append(bass2jax.partition_id_tensor())
        outs = bass2jax._bass_exec_p.bind(
            *operands,
            out_avals=tuple(out_avals),
            in_names=tuple(all_names),
            out_names=tuple(out_names),
            lowering_input_output_aliases=(),
            sim_require_finite=True,
            sim_require_nnan=True,
            nc=nc,
        )
        return tuple(outs)

    devices = jax.devices()[:N_CORES]
    mesh = Mesh(np.asarray(devices), ("core",))
    # 'at' is identical on every core: replicate instead of sharding 8 copies
    repl = [i for i, nm in enumerate(in_names) if nm == "at"]
    in_specs = tuple(
        PartitionSpec() if i in repl else PartitionSpec("core")
        for i in range(n_params)
    ) + (PartitionSpec("core"),) * n_outs
    out_specs = (PartitionSpec("core"),) * n_outs
    donate = tuple(range(n_params, n_params + n_outs))
    sharded = jax.jit(
        shard_map(_body, mesh=mesh, in_specs=in_specs, out_specs=out_specs,
                  check_rep=False),
        donate_argnums=donate, keep_unused=True,
    )

    def run(per_core_inputs):
        concat_in = [
            per_core_inputs[0][i] if i in repl else
            np.concatenate([per_core_inputs[c][i] for c in range(N_CORES)],
                           axis=0)
            for i in range(n_params)
        ]
        concat_zeros = [
            np.zeros((N_CORES * z.shape[0], *z.shape[1:]), z.dtype)
            for z in zero_outs
        ]
        out_arrs = sharded(*concat_in, *concat_zeros)
        return [
            np.asarray(out_arrs[i]).reshape(N_CORES, *out_avals[i].shape)
            for i in range(n_outs)
        ], out_names, in_names

    _CACHE["runner"] = run
    return run


def kernel(x, T, trans, emis, prior):
    x = np.asarray(x).astype(np.int64)
    T = np.asarray(T).astype(np.int64)
    trans = np.asarray(trans, dtype=np.float32)
    emis = np.ascontiguousarray(np.asarray(emis, dtype=np.float32))
    prior = np.asarray(prior, dtype=np.float32)

    if "main" not in _CACHE:
        _CACHE["main"] = _build_main_kernel()
    run = _get_runner()

    ln2 = np.log(2.0)
    fp8 = ml_dtypes.float8_e4m3fn
    # ---- host prep ----
    # single big exp pass over emis, shared by d and the token gather
    E = np.exp(emis)                                           # [N, M] fp32
    dsum = E.sum(axis=1, dtype=np.float32)                     # [N] = exp(d)

    # A' = diag(2^S_EP / dsum) @ softmax(trans, 0): the per-state emission
    # normalizer rides on the transition matrix, so ep stays raw exp(em).
    tm = trans.max(axis=0)
    ex = np.exp(trans - tm)
    ex /= ex.sum(axis=0)
    ex *= (np.float32(2.0 ** (S_EP + S_A)) / dsum)[:, None]
    at = np.ascontiguousarray(ex.T).astype(fp8)                # [k, j] fp8

    logpi = prior.astype(np.float64) - np.log(np.exp(prior, dtype=np.float32)
                                              .sum(dtype=np.float64))
    alpha0 = np.log(np.take(E, x[:, 0], axis=1)).astype(np.float64) \
        - np.log(dsum).astype(np.float64)[:, None] + logpi[:, None]
    a0m = alpha0.max(axis=0)
    m0 = np.log(np.exp(alpha0 - a0m).sum(axis=0)) + a0m        # [B]
    q0 = np.exp(alpha0 - m0).astype(np.float32)                # [N, B]

    # one global gather (faster than 8 per-core takes), then cast + slice
    ep8 = np.take(E, x.reshape(-1), axis=1).astype(fp8) \
        .reshape(N, B, TMAX)                                   # [N, B, TMAX] fp8
    ins = []
    for c in range(N_CORES):
        bs = slice(c * BL, (c + 1) * BL)
        epc = np.ascontiguousarray(ep8[:, bs, :]).reshape(N, BL * TMAX)
        q0c = np.ascontiguousarray(
            q0[:, bs].reshape(NT, 128, BL).transpose(1, 0, 2)
            .reshape(128, NT * BL)).astype(ml_dtypes.bfloat16)
        ins.append([at, epc, q0c])   # order must match in_names (at, ep, q0)

    import time as _time
    _t0 = _time.perf_counter_ns()
    outs, out_names, in_names = run(ins)
    _t1 = _time.perf_counter_ns()
    global LAST_EXEC_NS
    LAST_EXEC_NS = _t1 - _t0
    assert in_names == ["at", "ep", "q0"], in_names
    sig_all = outs[out_names.index("sig")]                     # [8, 1, 8160]

    # ---- host post (vectorized): m[b, t] from sigma history ----
    S = sig_all.reshape(N_CORES, NSTEP, BL).astype(np.float64)
    S = np.moveaxis(S, 0, 1).reshape(NSTEP, B)                 # [255, B] (c*BL+b)
    logS = np.log(S)
    nblk = NSTEP // KRE                                        # 31 full blocks
    rlog = logS[KRE - 1::KRE][:nblk]                           # renorm rows
    blockcum = np.concatenate(
        [np.zeros((1, B)), np.cumsum(rlog - KRE * C_EFF * ln2, axis=0)])
    ii = np.arange(NSTEP)
    m = np.empty((TMAX, B))
    m[0] = m0
    m[1:] = m0[None, :] + blockcum[ii // KRE] + logS[ii] \
        - C_EFF * ln2 * ((ii % KRE) + 1)[:, None]
    out = m[T - 1, np.arange(B)].astype(np.float32)[:, None]
    return out
